# revision 8
# baseline (speedup 1.0000x reference)
"""Cross-Invariant-Point-Attention TRN2 kernel.

Sharding: 8 cores = 4 batches x 2 head-halves (6 heads each). Each core
computes its (batch, head-half) attention fully on-chip; the host sums the
two s_upd partials per batch (Wout is row-split by head) and concatenates
the per-half attention outputs. No device collectives.

Per-head feature vectors are augmented to 30 dims so every softmax bias
term rides inside the QK matmul:
  Qf = [alpha*q ; 2*gam_h*q_pts ; -gam_h ; -gam_h*|q_pts|^2 (- ln denom)]
  Kf = [k       ; k_pts         ; |k_pts|^2 ; 1]
Pass 1 computes S = Qf.Kf row-major, exp with accum_out -> denominators
(softmax needs no max-subtraction: logits are bounded), and writes the
normalized attention output. Pass 2 folds -ln(denom) into Qf and computes
exp(S^T) = normalized a^T directly, which feeds o = a @ v with v as the
stationary operand - the [512,512] attention matrix is never transposed.

Self-contained: hardcoded shapes, no sibling imports.
"""

import math

import numpy as np

import concourse.bass as bass
import concourse.tile as tile
from concourse import bacc, mybir
from concourse.bass_utils import run_bass_kernel_spmd
from concourse.masks import make_identity

F32 = mybir.dt.float32
AF = mybir.ActivationFunctionType
OP = mybir.AluOpType
AX = mybir.AxisListType

B, LD, LS, CS, CH, H, PQ, PV = 4, 512, 512, 384, 16, 12, 4, 8
NH = H // 2              # local heads per core
EPS = 1e-8
W_C = math.sqrt(2.0 / (9.0 * PQ))
W_L = math.sqrt(0.5)
ALPHA = W_L / math.sqrt(CH)
N_CORES = 8
NT = LD // 128           # 4 token tiles
KT = CS // 128           # 3 contraction tiles
FD = 32                  # feature dims per head (16+12+2 bias, 2 zero pad)
VD = 40                  # vaug dims per head (24 v_pts + 16 v)


def _emit(tc, din, a_out, sup):
    nc = tc.nc
    from contextlib import ExitStack
    ctx = ExitStack()
    with ctx:
        persist = ctx.enter_context(tc.tile_pool(name="persist", bufs=1))
        work = ctx.enter_context(tc.tile_pool(name="work", bufs=4))
        epool = ctx.enter_context(tc.tile_pool(name="epool", bufs=4))
        atpool = ctx.enter_context(tc.tile_pool(name="atpool", bufs=8))
        spool = ctx.enter_context(tc.tile_pool(name="spool", bufs=2))
        psS = ctx.enter_context(tc.tile_pool(name="psS", bufs=2, space="PSUM"))
        ptr = ctx.enter_context(tc.tile_pool(name="ptr", bufs=2, space="PSUM"))
        pm = ctx.enter_context(tc.tile_pool(name="pm", bufs=2, space="PSUM"))
        pot = ctx.enter_context(tc.tile_pool(name="pot", bufs=2, space="PSUM"))

        ts = bass.ts

        # ---- constants / inputs to SBUF ----
        ident = persist.tile([128, 128], F32, tag="ident")
        make_identity(nc, ident)
        eps_t = persist.tile([128, 1], F32, tag="eps_t")
        nc.vector.memset(eps_t[:], EPS)

        def load(name, shape, pat, **kw):
            t = persist.tile(shape, F32, tag=name)
            nc.sync.dma_start(t[:], din[name].rearrange(pat, **kw))
            return t

        def load_bcast(name, cols):
            t = persist.tile([128, cols], F32, tag=name)
            src = din[name]
            nc.sync.dma_start(
                t[:], bass.AP(tensor=src.tensor, offset=src.offset,
                              ap=[[0, 128]] + list(src.ap[1:])))
            return t

        sdT = load("sdT", [128, KT, LD], "(t p) n -> p t n", p=128)
        ssT = load("ssT", [128, KT, LS], "(t p) n -> p t n", p=128)
        Wd = load("Wd", [128, KT, 168], "(t p) n -> p t n", p=128)
        Ws = load("Ws", [128, KT, 408], "(t p) n -> p t n", p=128)
        raugd = load("raugd", [128, NT, 16], "(t p) c -> p t c", p=128)
        raugs = load("raugs", [128, NT, 16], "(t p) c -> p t c", p=128)
        btile = load_bcast("btile", 72)
        gamrow = load_bcast("gamrow", NH)     # holds -gam_h

        Wo = persist.tile([96, CS], F32, tag="Wo")
        nc.sync.dma_start(Wo[:], din["Wo"][:])
        Wx = persist.tile([128, CS], F32, tag="Wx")
        nc.sync.dma_start(Wx[:], din["Wx"][:])
        Wn = persist.tile([64, CS], F32, tag="Wn")
        nc.sync.dma_start(Wn[:], din["Wn"][:])

        # ---- persistent intermediates ----
        dstfeat = persist.tile([128, NT, 168], F32, tag="dstfeat")
        srcfeat = persist.tile([128, NT, 408], F32, tag="srcfeat")
        qp = persist.tile([128, NT, 72], F32, tag="qp")
        kvp = persist.tile([128, NT, 216], F32, tag="kvp")
        q2r = persist.tile([128, NT, NH], F32, tag="q2r")
        q2g = persist.tile([128, NT, NH], F32, tag="q2g")
        k2r = persist.tile([128, NT, NH], F32, tag="k2r")
        den = persist.tile([128, NT, NH], F32, tag="den")
        rden = persist.tile([128, NT, NH], F32, tag="rden")
        lnd = persist.tile([128, NT, NH], F32, tag="lnd")
        qfeat = persist.tile([128, NT, NH * FD], F32, tag="qfeat")
        kfeat = persist.tile([128, NT, NH * FD], F32, tag="kfeat")
        vaug = persist.tile([128, NT, NH * VD], F32, tag="vaug")
        sqq = persist.tile([128, NT, 72], F32, tag="sqq")
        ksq = persist.tile([128, NT, 72], F32, tag="ksq")
        qfT = [persist.tile([96, LD], F32, tag=f"qfT{i}", name=f"qfT{i}") for i in range(2)]
        kfT = [persist.tile([96, LS], F32, tag=f"kfT{i}", name=f"kfT{i}") for i in range(2)]
        qfT2 = [persist.tile([96, LD], F32, tag=f"qfT2{i}", name=f"qfT2{i}") for i in range(2)]
        oT = [persist.tile([VD, LD], F32, tag=f"oT{h}", name=f"oT{h}") for h in range(NH)]
        oGrp = persist.tile([96, LD], F32, tag="oGrp")
        optraw = persist.tile([128, NT, 144], F32, tag="optraw")
        optfin = persist.tile([128, NT, 192], F32, tag="optfin")
        optT_a = persist.tile([128, LD], F32, tag="optT_a")
        optT_b = persist.tile([64, LD], F32, tag="optT_b")

        # ---- projections:  feat^T(tokens) = (s^T)^T @ W  per token tile ----
        for it in range(NT):
            pd = pm.tile([128, 408], F32, tag="pm")
            for kt in range(KT):
                nc.tensor.matmul(pd[:, 0:168], sdT[:, kt, ts(it, 128)],
                                 Wd[:, kt, :], start=(kt == 0),
                                 stop=(kt == KT - 1))
            nc.vector.tensor_copy(dstfeat[:, it, :], pd[:, 0:168])
            ps = pm.tile([128, 408], F32, tag="pm")
            for kt in range(KT):
                nc.tensor.matmul(ps[:], ssT[:, kt, ts(it, 128)],
                                 Ws[:, kt, :], start=(kt == 0),
                                 stop=(kt == KT - 1))
            nc.vector.tensor_copy(srcfeat[:, it, :], ps[:])

        # ---- rigid apply: pts = R @ praw + t  (per token, per coord) ----
        for it in range(NT):
            for d in range(3):
                o = qp[:, it, d * 24:(d + 1) * 24]
                nc.vector.tensor_scalar(
                    o, dstfeat[:, it, 96:120],
                    raugd[:, it, 3 * d:3 * d + 1],
                    raugd[:, it, 9 + d:10 + d], OP.mult, OP.add)
                for e in (1, 2):
                    nc.vector.scalar_tensor_tensor(
                        o, dstfeat[:, it, 96 + e * 24:120 + e * 24],
                        raugd[:, it, 3 * d + e:3 * d + e + 1], o,
                        OP.mult, OP.add)
                o = kvp[:, it, d * 72:(d + 1) * 72]
                nc.vector.tensor_scalar(
                    o, srcfeat[:, it, 192:264],
                    raugs[:, it, 3 * d:3 * d + 1],
                    raugs[:, it, 9 + d:10 + d], OP.mult, OP.add)
                for e in (1, 2):
                    nc.vector.scalar_tensor_tensor(
                        o, srcfeat[:, it, 192 + e * 72:264 + e * 72],
                        raugs[:, it, 3 * d + e:3 * d + e + 1], o,
                        OP.mult, OP.add)

        # ---- squared point norms per head ----
        nc.vector.tensor_mul(sqq[:], qp[:], qp[:])
        for it in range(NT):
            nc.vector.tensor_reduce(
                q2r[:, it, :],
                sqq[:, it, :].rearrange("p (d h q) -> p h d q",
                                        d=3, h=NH, q=PQ),
                axis=AX.XY, op=OP.add)
            kv = kvp[:, it, :].rearrange("p (d h q) -> p h d q",
                                         d=3, h=NH, q=PQ + PV)
            nc.vector.tensor_mul(
                ksq[:, it, :].rearrange("p (h d q) -> p h d q",
                                        h=NH, d=3, q=PQ),
                kv[:, :, :, 0:PQ], kv[:, :, :, 0:PQ])
            nc.vector.tensor_reduce(
                k2r[:, it, :],
                ksq[:, it, :].rearrange("p (h dq) -> p h dq", h=NH),
                axis=AX.X, op=OP.add)

        # ---- assemble qfeat / kfeat / vaug ----
        qf = qfeat.rearrange("p t (h f) -> p t h f", h=NH)
        kf = kfeat.rearrange("p t (h f) -> p t h f", h=NH)
        va = vaug.rearrange("p t (h f) -> p t h f", h=NH)
        nc.vector.tensor_copy(
            qf[:, :, :, 0:16],
            dstfeat[:, :, 0:96].rearrange("p t (h c) -> p t h c", h=NH))
        nc.vector.tensor_copy(
            kf[:, :, :, 0:16],
            srcfeat[:, :, 0:96].rearrange("p t (h c) -> p t h c", h=NH))
        nc.vector.tensor_copy(
            va[:, :, :, 24:40],
            srcfeat[:, :, 96:192].rearrange("p t (h c) -> p t h c", h=NH))
        bt = btile.rearrange("p (d h q) -> p h d q", d=3, h=NH, q=PQ)
        for it in range(NT):
            nc.vector.tensor_mul(
                qf[:, it, :, 16:28].rearrange("p h (d q) -> p h d q",
                                              d=3, q=PQ),
                qp[:, it, :].rearrange("p (d h q) -> p h d q",
                                       d=3, h=NH, q=PQ),
                bt)
            kv = kvp[:, it, :].rearrange("p (d h q) -> p h d q",
                                         d=3, h=NH, q=PQ + PV)
            nc.vector.tensor_copy(
                kf[:, it, :, 16:28].rearrange("p h (d q) -> p h d q",
                                              d=3, q=PQ),
                kv[:, :, :, 0:PQ])
            nc.vector.tensor_copy(
                va[:, it, :, 0:24].rearrange("p h (d q) -> p h d q",
                                              d=3, q=PV),
                kv[:, :, :, PQ:PQ + PV])
        for hl in range(NH):
            nc.vector.tensor_scalar(
                q2g[:, :, hl:hl + 1], q2r[:, :, hl:hl + 1],
                gamrow[:, hl:hl + 1], None, OP.mult)
            for it in range(NT):
                nc.vector.tensor_scalar(
                    qf[:, it, hl, 28:29], gamrow[:, hl:hl + 1],
                    1.0, None, OP.mult)
        nc.vector.tensor_copy(qf[:, :, :, 29], q2g[:])
        nc.vector.tensor_copy(kf[:, :, :, 28], k2r[:])
        nc.vector.memset(kf[:, :, :, 29], 1.0)
        nc.vector.memset(qf[:, :, :, 30:32], 0.0)
        nc.vector.memset(kf[:, :, :, 30:32], 0.0)

        # ---- transpose feature blocks to [feat, tokens] ----
        def transpose_feats(src_t, dsts):
            for it in range(NT):
                for half in range(2):
                    p = ptr.tile([128, 128], F32, tag="tr")
                    nc.tensor.transpose(
                        p[0:96, :], src_t[:, it, half * 96:half * 96 + 96],
                        ident[:])
                    nc.vector.tensor_copy(dsts[half][:, ts(it, 128)],
                                          p[0:96, :])

        transpose_feats(qfeat, qfT)
        transpose_feats(kfeat, kfT)

        # ---- pass 1: S row-major; exp; denom; normalized attention out ----
        for hl in range(NH):
            half, hh = hl // 3, (hl % 3) * FD
            for it in range(NT):
                S = psS.tile([128, 512], F32, tag="s")
                nc.tensor.matmul(S[:], qfT[half][hh:hh + FD, ts(it, 128)],
                                 kfT[half][hh:hh + FD, :],
                                 start=True, stop=True)
                e = epool.tile([128, 512], F32, tag="e")
                nc.scalar.activation(e[:], S[:], AF.Exp,
                                     accum_out=den[:, it, hl:hl + 1])
                nc.vector.reciprocal(rden[:, it, hl:hl + 1],
                                     den[:, it, hl:hl + 1])
                nc.scalar.activation(lnd[:, it, hl:hl + 1],
                                     den[:, it, hl:hl + 1], AF.Ln)
                nc.vector.tensor_scalar(e[:], e[:], rden[:, it, hl:hl + 1],
                                        None, OP.mult)
                nc.sync.dma_start(a_out[hl, it * 128:(it + 1) * 128, :], e[:])

        # ---- fold -ln(denom) into Qf bias col; re-transpose ----
        nc.vector.tensor_sub(qf[:, :, :, 29], q2g[:], lnd[:])
        transpose_feats(qfeat, qfT2)

        # ---- pass 2: a^T = exp(S^T - ln d);  o^T = (v|v_pts)^T . a^T ----
        for hl in range(NH):
            half, hh = hl // 3, (hl % 3) * FD
            ats = []
            for jt in range(NT):
                St = psS.tile([128, 512], F32, tag="s")
                nc.tensor.matmul(St[:], kfT[half][hh:hh + FD, ts(jt, 128)],
                                 qfT2[half][hh:hh + FD, :],
                                 start=True, stop=True)
                at = atpool.tile([128, 512], F32, tag="at")
                nc.scalar.activation(at[:], St[:], AF.Exp)
                ats.append(at)
            ot = pot.tile([VD, 512], F32, tag="ot")
            for jt in range(NT):
                nc.tensor.matmul(ot[:], vaug[:, jt, hl * VD:(hl + 1) * VD],
                                 ats[jt][:], start=(jt == 0),
                                 stop=(jt == NT - 1))
            nc.vector.tensor_copy(oT[hl][:], ot[:])

        # ---- regroup o channel rows (partition move -> DMA) ----
        for hl in range(NH):
            nc.sync.dma_start(oGrp[hl * 16:(hl + 1) * 16, :], oT[hl][24:40, :])

        # ---- point outputs: back to token-major, inverse rigid, norms ----
        for hl in range(NH):
            for it in range(NT):
                p = ptr.tile([128, 128], F32, tag="tr")
                nc.tensor.transpose(p[:, 0:24], oT[hl][0:24, ts(it, 128)],
                                    ident[0:24, 0:24])
                nc.vector.tensor_copy(optraw[:, it, hl * 24:(hl + 1) * 24],
                                      p[:, 0:24])
        orv = optraw.rearrange("p t (h e q) -> p t h e q", h=NH, e=3, q=PV)
        for it in range(NT):
            for d in range(3):
                o = optfin[:, it, d * 48:(d + 1) * 48].rearrange(
                    "p (h q) -> p h q", h=NH)
                nc.vector.tensor_scalar(
                    o, orv[:, it, :, 0, :],
                    raugd[:, it, d:d + 1],
                    raugd[:, it, 12 + d:13 + d], OP.mult, OP.subtract)
                for e in (1, 2):
                    nc.vector.scalar_tensor_tensor(
                        o, orv[:, it, :, e, :],
                        raugd[:, it, 3 * e + d:3 * e + d + 1], o,
                        OP.mult, OP.add)
            n1 = work.tile([128, 48], F32, tag="n1")
            n2 = work.tile([128, 48], F32, tag="n2")
            nc.vector.tensor_mul(n1[:], optfin[:, it, 0:48],
                                 optfin[:, it, 0:48])
            nc.vector.tensor_mul(n2[:], optfin[:, it, 48:96],
                                 optfin[:, it, 48:96])
            nc.vector.tensor_add(n1[:], n1[:], n2[:])
            nc.vector.tensor_mul(n2[:], optfin[:, it, 96:144],
                                 optfin[:, it, 96:144])
            nc.vector.tensor_add(n1[:], n1[:], n2[:])
            nc.scalar.activation(optfin[:, it, 144:192], n1[:], AF.Sqrt,
                                 bias=eps_t[:])

        # ---- transpose point outputs for the final projection ----
        for it in range(NT):
            p = ptr.tile([128, 128], F32, tag="tr")
            nc.tensor.transpose(p[:], optfin[:, it, 0:128], ident[:])
            nc.vector.tensor_copy(optT_a[:, ts(it, 128)], p[:])
            p = ptr.tile([128, 128], F32, tag="tr")
            nc.tensor.transpose(p[0:64, :], optfin[:, it, 128:192], ident[:])
            nc.vector.tensor_copy(optT_b[:, ts(it, 128)], p[0:64, :])

        # ---- final projection: s_upd partial = cat^T chunks @ Wout rows ----
        for it in range(NT):
            sp = pm.tile([128, 408], F32, tag="pm")
            nc.tensor.matmul(sp[:, 0:CS], oGrp[:, ts(it, 128)], Wo[:],
                             start=True, stop=False)
            nc.tensor.matmul(sp[:, 0:CS], optT_a[:, ts(it, 128)], Wx[:],
                             start=False, stop=False)
            nc.tensor.matmul(sp[:, 0:CS], optT_b[:, ts(it, 128)], Wn[:],
                             start=False, stop=True)
            sb = spool.tile([128, CS], F32, tag="sb")
            nc.vector.tensor_copy(sb[:], sp[:, 0:CS])
            nc.sync.dma_start(sup[it * 128:(it + 1) * 128, :], sb[:])


def _build():
    nc = bacc.Bacc("TRN2", target_bir_lowering=False, debug=False,
                   num_devices=N_CORES)
    din = {}
    for name, shape in [
        ("sdT", [CS, LD]), ("ssT", [CS, LS]),
        ("Wd", [CS, 168]), ("Ws", [CS, 408]),
        ("raugd", [LD, 16]), ("raugs", [LS, 16]),
        ("btile", [1, 72]), ("gamrow", [1, NH]),
        ("Wo", [96, CS]), ("Wx", [128, CS]), ("Wn", [64, CS]),
    ]:
        din[name] = nc.dram_tensor(name, shape, F32, kind="ExternalInput").ap()
    a_out = nc.dram_tensor("a_out", [NH, LD, LS], F32,
                           kind="ExternalOutput").ap()
    sup = nc.dram_tensor("sup", [LD, CS], F32, kind="ExternalOutput").ap()

    with tile.TileContext(nc) as tc:
        _emit(tc, din, a_out, sup)
    nc.compile()
    return nc


_NC = None


def _get_nc():
    global _NC
    if _NC is None:
        _NC = _build()
    return _NC


def _prep_core(inputs, b, g):
    hs = slice(g * NH, (g + 1) * NH)
    hw = np.log1p(np.exp(inputs["head_weights"].astype(np.float64)))[hs]
    gam = (0.5 * W_L * W_C * hw).astype(np.float32)           # [NH]

    Wq = inputs["Wq"][:, g * NH * CH:(g + 1) * NH * CH] * ALPHA
    Wkv = inputs["Wkv"]
    idx_k = np.array([h * 2 * CH + c
                      for h in range(g * NH, (g + 1) * NH)
                      for c in range(CH)])
    Wk = Wkv[:, idx_k]
    Wv = Wkv[:, idx_k + CH]

    idx_qp = np.array([d * H * PQ + h * PQ + p
                       for d in range(3) for h in range(g * NH, (g + 1) * NH)
                       for p in range(PQ)])
    Wqp = inputs["Wqp"][:, idx_qp]                            # [CS, 72]
    npts = PQ + PV
    idx_kvp = np.array([d * H * npts + h * npts + p
                        for d in range(3) for h in range(g * NH, (g + 1) * NH)
                        for p in range(npts)])
    Wkvp = inputs["Wkvp"][:, idx_kvp]                         # [CS, 216]

    R_d, t_d = inputs["R_dst"][b], inputs["t_dst"][b]
    R_s, t_s = inputs["R_src"][b], inputs["t_src"][b]
    tinv = np.einsum("led,le->ld", R_d, t_d)                  # R^T t
    raugd = np.concatenate([R_d.reshape(LD, 9), t_d, tinv,
                            np.zeros((LD, 1), np.float32)], axis=1)
    raugs = np.concatenate([R_s.reshape(LS, 9), t_s,
                            np.zeros((LS, 4), np.float32)], axis=1)

    beta = 2.0 * gam
    btile = np.tile(beta[None, :, None], (3, 1, PQ)).reshape(1, 72)
    gamrow = (-gam)[None, :]

    Wout = inputs["Wout"]
    rows_o = np.arange(g * NH * CH, (g + 1) * NH * CH)
    blk = H * PV
    rows_pts = np.concatenate([H * CH + k * blk + g * NH * PV
                               + np.arange(NH * PV) for k in range(4)])
    Wpts = Wout[rows_pts]                                     # [192, CS]

    f32 = np.float32
    return {
        "sdT": np.ascontiguousarray(inputs["s_dst"][b].T, dtype=f32),
        "ssT": np.ascontiguousarray(inputs["s_src"][b].T, dtype=f32),
        "Wd": np.ascontiguousarray(np.concatenate([Wq, Wqp], 1), dtype=f32),
        "Ws": np.ascontiguousarray(np.concatenate([Wk, Wv, Wkvp], 1),
                                   dtype=f32),
        "raugd": np.ascontiguousarray(raugd, dtype=f32),
        "raugs": np.ascontiguousarray(raugs, dtype=f32),
        "btile": np.ascontiguousarray(btile, dtype=f32),
        "gamrow": np.ascontiguousarray(gamrow, dtype=f32),
        "Wo": np.ascontiguousarray(Wout[rows_o], dtype=f32),
        "Wx": np.ascontiguousarray(Wpts[0:128], dtype=f32),
        "Wn": np.ascontiguousarray(Wpts[128:192], dtype=f32),
    }


def kernel(**inputs):
    nc = _get_nc()
    in_maps = [_prep_core(inputs, c // 2, c % 2) for c in range(N_CORES)]
    res = run_bass_kernel_spmd(nc, in_maps, list(range(N_CORES)))

    s_upd = np.zeros((B, LD, CS), np.float32)
    a = np.zeros((B, H, LD, LS), np.float32)
    for c in range(N_CORES):
        b, g = c // 2, c % 2
        s_upd[b] += res.results[c]["sup"]
        a[b, g * NH:(g + 1) * NH] = res.results[c]["a_out"]
    s_upd += np.asarray(inputs["b_out"], np.float32)[None, None, :]
    return s_upd, a


# revision 35
# speedup vs baseline: 1.6199x; 1.6199x over previous
"""Cross-Invariant-Point-Attention TRN2 kernel.

Sharding: 8 cores = 4 batches x 2 head-halves (6 heads each). Each core
computes its (batch, head-half) attention fully on-chip; the host sums the
two s_upd partials per batch (Wout is row-split by head) and concatenates
the per-half attention outputs. No device collectives.

Per-head feature vectors are augmented to 30 dims so every softmax bias
term rides inside the QK matmul:
  Qf = [alpha*q ; 2*gam_h*q_pts ; -gam_h ; -gam_h*|q_pts|^2 (- ln denom)]
  Kf = [k       ; k_pts         ; |k_pts|^2 ; 1]
Pass 1 computes S = Qf.Kf row-major, exp with accum_out -> denominators
(softmax needs no max-subtraction: logits are bounded), and writes the
normalized attention output. Pass 2 folds -ln(denom) into Qf and computes
exp(S^T) = normalized a^T directly, which feeds o = a @ v with v as the
stationary operand - the [512,512] attention matrix is never transposed.

Self-contained: hardcoded shapes, no sibling imports.
"""

import math

import numpy as np

import concourse.bass as bass
import concourse.tile as tile
from concourse import bacc, mybir
from concourse.bass_utils import run_bass_kernel_spmd
from concourse.masks import make_identity

F32 = mybir.dt.float32
F32R = mybir.dt.float32r
AF = mybir.ActivationFunctionType
OP = mybir.AluOpType
AX = mybir.AxisListType

B, LD, LS, CS, CH, H, PQ, PV = 4, 512, 512, 384, 16, 12, 4, 8
NH = H // 2              # local heads per core
EPS = 1e-8
W_C = math.sqrt(2.0 / (9.0 * PQ))
W_L = math.sqrt(0.5)
ALPHA = W_L / math.sqrt(CH)
N_CORES = 8
NT = LD // 128           # 4 token tiles
KT = CS // 128           # 3 contraction tiles
FD = 32                  # feature dims per head (16+12+2 bias, 2 zero pad)
VD = 42                  # per head: 24 v_pts + 16 v + ones row + zero pad


def _emit(tc, din, a_out, sup):
    nc = tc.nc
    from contextlib import ExitStack
    ctx = ExitStack()
    with ctx:
        persist = ctx.enter_context(tc.tile_pool(name="persist", bufs=1))
        work = ctx.enter_context(tc.tile_pool(name="work", bufs=4))
        epool = ctx.enter_context(tc.tile_pool(name="epool", bufs=3))
        atpool = ctx.enter_context(tc.tile_pool(name="atpool", bufs=8))
        psS = ctx.enter_context(tc.tile_pool(name="psS", bufs=2, space="PSUM"))
        pm = ctx.enter_context(tc.tile_pool(name="pm", bufs=4, space="PSUM"))
        ptr = pm
        pot = pm

        ts = bass.ts
        FR = mybir.dt.float32r

        def r(ap):
            return ap.bitcast(FR)

        # ---- constants / inputs to SBUF (DMAs spread across engine queues,
        # src-side first: the k feature path is the critical path) ----
        ident = persist.tile([128, 128], F32, tag="ident")
        make_identity(nc, ident)
        eps_t = persist.tile([128, 1], F32, tag="eps_t")
        nc.gpsimd.memset(eps_t[:], EPS)
        ones_t = persist.tile([128, NH], F32, tag="ones_t")
        nc.gpsimd.memset(ones_t[:], 1.0)

        def load(eng, name, shape, pat, dt=F32, **kw):
            t = persist.tile(shape, dt, tag=name)
            eng.dma_start(t[:], din[name].rearrange(pat, **kw))
            return t

        def load_bcast(name, cols):
            t = persist.tile([128, cols], F32, tag=name)
            src = din[name]
            nc.gpsimd.dma_start(
                t[:], bass.AP(tensor=src.tensor, offset=src.offset,
                              ap=[[0, 128]] + list(src.ap[1:])))
            return t

        Ws = load(nc.gpsimd, "Ws", [128, KT, 408], "(t p) n -> p t n", dt=F32R, p=128)
        Wd = load(nc.gpsimd, "Wd", [128, KT, 256], "(t p) n -> p t n", dt=F32R, p=128)
        ssT = persist.tile([128, KT, LS], F32R, tag="ssT")
        sdT = persist.tile([128, KT, LD], F32R, tag="sdT")
        for it in range(NT):
            nc.sync.dma_start(
                ssT[:, :, ts(it, 128)],
                din["ssT"].rearrange("(t p) n -> p t n", p=128)
                [:, :, ts(it, 128)])
        for it in range(NT):
            nc.scalar.dma_start(
                sdT[:, :, ts(it, 128)],
                din["sdT"].rearrange("(t p) n -> p t n", p=128)
                [:, :, ts(it, 128)])
        raugs = load(nc.gpsimd, "raugs", [128, NT, 16], "(t p) c -> p t c",
                     p=128)
        raugd = load(nc.gpsimd, "raugd", [128, NT, 16], "(t p) c -> p t c",
                     p=128)
        btile = load_bcast("btile", 72)
        gamrow = load_bcast("gamrow", NH)     # holds -gam_h

        Wo = persist.tile([96, CS], F32R, tag="Wo")
        nc.sync.dma_start(Wo[:], din["Wo"][:])
        Wx = persist.tile([96, CS], F32R, tag="Wx")
        nc.sync.dma_start(Wx[:], din["Wx"][:])
        Wn = persist.tile([96, CS], F32R, tag="Wn")
        nc.sync.dma_start(Wn[:], din["Wn"][:])

        # ---- persistent intermediates ----
        dstfeat = persist.tile([128, NT, 168], F32, tag="dstfeat")
        srcfeat = persist.tile([128, NT, 408], F32, tag="srcfeat")
        qp = persist.tile([128, NT, 72], F32, tag="qp")
        kvp = persist.tile([128, NT, 216], F32, tag="kvp")
        q2r = persist.tile([128, NT, NH], F32, tag="q2r")
        q2g = persist.tile([128, NT, NH], F32, tag="q2g")
        k2r = persist.tile([128, NT, NH], F32, tag="k2r")
        den = persist.tile([128, NT, NH], F32, tag="den")
        rden = persist.tile([128, NT, NH], F32, tag="rden")
        qfeat = persist.tile([128, NT, NH * FD], F32, tag="qfeat")
        kfeat = persist.tile([128, NT, NH * FD], F32, tag="kfeat")
        vaug = persist.tile([128, NT, NH * VD], F32R, tag="vaug")
        sqq = persist.tile([128, NT, 72], F32, tag="sqq")
        ksq = persist.tile([128, NT, 72], F32, tag="ksq")
        qfT = [persist.tile([96, LD], F32R, tag=f"qfT{i}", name=f"qfT{i}")
               for i in range(2)]
        kfT = [persist.tile([96, LS], F32R, tag=f"kfT{i}", name=f"kfT{i}")
               for i in range(2)]
        oT_all = persist.tile([VD, NH, LD], F32, tag="oT_all")
        oGrp = persist.tile([96, LD], F32R, tag="oGrp")
        optfin = persist.tile([128, NT, 192], F32, tag="optfin")
        optT_a = persist.tile([96, LD], F32R, tag="optT_a")
        optT_b = persist.tile([96, LD], F32R, tag="optT_b")

        qf = qfeat.rearrange("p t (h f) -> p t h f", h=NH)
        kf = kfeat.rearrange("p t (h f) -> p t h f", h=NH)
        va = vaug.rearrange("p t (h f) -> p t h f", h=NH)

        def transpose_feats(src_t, dsts, its=range(NT)):
            for it in its:
                for half in range(2):
                    p = ptr.tile([128, 128], F32, tag="pm")
                    nc.tensor.transpose(
                        p[0:96, :], src_t[:, it, half * 96:half * 96 + 96],
                        ident[:])
                    nc.vector.tensor_copy(dsts[half][:, ts(it, 128)],
                                          p[0:96, :])

        # ================= SRC (k/v) side first =================
        for it in range(NT):
            ps = pm.tile([128, 512], F32, tag="pm")
            for kt in range(KT):
                nc.tensor.matmul(ps[:, 0:408], r(ssT[:, kt, ts(it, 128)]),
                                 r(Ws[:, kt, :]), start=(kt == 0),
                                 stop=(kt == KT - 1))
            nc.scalar.copy(srcfeat[:, it, :], ps[:, 0:408])
            # rigid apply on gpsimd (overlaps q-side DVE work later)
            for d in range(3):
                o = kvp[:, it, d * 72:(d + 1) * 72]
                nc.vector.tensor_scalar(
                    o, srcfeat[:, it, 192:264],
                    raugs[:, it, 3 * d:3 * d + 1],
                    raugs[:, it, 9 + d:10 + d], OP.mult, OP.add)
                for e in (1, 2):
                    nc.vector.scalar_tensor_tensor(
                        o, srcfeat[:, it, 192 + e * 72:264 + e * 72],
                        raugs[:, it, 3 * d + e:3 * d + e + 1], o,
                        OP.mult, OP.add)
            kv = kvp[:, it, :].rearrange("p (d h q) -> p h d q",
                                         d=3, h=NH, q=PQ + PV)
            nc.vector.tensor_mul(
                ksq[:, it, :].rearrange("p (h d q) -> p h d q",
                                        h=NH, d=3, q=PQ),
                kv[:, :, :, 0:PQ], kv[:, :, :, 0:PQ])
            nc.vector.tensor_reduce(
                k2r[:, it, :],
                ksq[:, it, :].rearrange("p (h dq) -> p h dq", h=NH),
                axis=AX.X, op=OP.add)
            nc.vector.tensor_copy(
                kf[:, it, :, 0:16],
                srcfeat[:, it, 0:96].rearrange("p (h c) -> p h c", h=NH))
            nc.vector.tensor_copy(
                kf[:, it, :, 16:28].rearrange("p h (d q) -> p h d q",
                                              d=3, q=PQ),
                kv[:, :, :, 0:PQ])
            nc.vector.tensor_copy(kf[:, it, :, 28], k2r[:, it, :])
            nc.gpsimd.memset(kf[:, it, :, 29], 1.0)
            nc.gpsimd.memset(kf[:, it, :, 30:32], 0.0)
            nc.scalar.copy(
                va[:, it, :, 24:40],
                srcfeat[:, it, 96:192].rearrange("p (h c) -> p h c", h=NH))
            nc.vector.tensor_copy(
                va[:, it, :, 0:24].rearrange("p h (d q) -> p h d q",
                                             d=3, q=PV),
                kv[:, :, :, PQ:PQ + PV])
            nc.vector.tensor_copy(va[:, it, :, 40], ones_t[:])
            nc.vector.tensor_scalar(va[:, it, :, 41], ones_t[:], 0.0, None,
                                    OP.mult)
            transpose_feats(kfeat, kfT, its=[it])

        # ================= DST (q) side =================
        bt = btile.rearrange("p (d h q) -> p h d q", d=3, h=NH, q=PQ)
        for it in range(NT):
            pd = pm.tile([128, 512], F32, tag="pm")
            for kt in range(KT):
                nc.tensor.matmul(pd[:, 0:256], r(sdT[:, kt, ts(it, 128)]),
                                 r(Wd[:, kt, :]), start=(kt == 0),
                                 stop=(kt == KT - 1))
            nc.scalar.copy(dstfeat[:, it, :], pd[:, 0:168])
            for d in range(3):
                o = qp[:, it, d * 24:(d + 1) * 24]
                nc.vector.tensor_scalar(
                    o, dstfeat[:, it, 96:120],
                    raugd[:, it, 3 * d:3 * d + 1],
                    raugd[:, it, 9 + d:10 + d], OP.mult, OP.add)
                for e in (1, 2):
                    nc.vector.scalar_tensor_tensor(
                        o, dstfeat[:, it, 96 + e * 24:120 + e * 24],
                        raugd[:, it, 3 * d + e:3 * d + e + 1], o,
                        OP.mult, OP.add)
            nc.vector.tensor_mul(sqq[:, it, :], qp[:, it, :], qp[:, it, :])
            nc.vector.tensor_reduce(
                q2r[:, it, :],
                sqq[:, it, :].rearrange("p (d h q) -> p h d q",
                                        d=3, h=NH, q=PQ),
                axis=AX.XY, op=OP.add)
            nc.vector.tensor_copy(
                qf[:, it, :, 0:16],
                dstfeat[:, it, 0:96].rearrange("p (h c) -> p h c", h=NH))
            nc.vector.tensor_mul(
                qf[:, it, :, 16:28].rearrange("p h (d q) -> p h d q",
                                              d=3, q=PQ),
                qp[:, it, :].rearrange("p (d h q) -> p h d q",
                                       d=3, h=NH, q=PQ),
                bt)
            for hl in range(NH):
                nc.vector.tensor_scalar(
                    q2g[:, it, hl:hl + 1], q2r[:, it, hl:hl + 1],
                    gamrow[:, hl:hl + 1], None, OP.mult)
                nc.vector.tensor_scalar(
                    qf[:, it, hl, 28:29], gamrow[:, hl:hl + 1],
                    1.0, None, OP.mult)
            nc.vector.tensor_copy(qf[:, it, :, 29], q2g[:, it, :])
            nc.gpsimd.memset(qf[:, it, :, 30:32], 0.0)
            transpose_feats(qfeat, qfT, its=[it])

        # ---- pass A: unnormalized a^T = exp(S^T); o^T_u = vaug^T . a^T ----
        # vaug carries a ones-row per head, so o^T_u row 40 is the softmax
        # denominator for free.
        for hl in range(NH):
            half, hh = hl // 3, (hl % 3) * FD
            ats = []
            for jp in range(NT // 2):
                St = psS.tile([128, 2, 512], F32, tag="s")
                for k in range(2):
                    jt = 2 * jp + k
                    nc.tensor.matmul(St[:, k, :],
                                     r(kfT[half][hh:hh + FD, ts(jt, 128)]),
                                     r(qfT[half][hh:hh + FD, :]),
                                     start=True, stop=True)
                at = atpool.tile([128, 2, 512], F32R, tag="at")
                nc.scalar.activation(at[:], St[:], AF.Exp)
                ats.append(at)
            ot = pot.tile([VD, 512], F32, tag="pm")
            for jt in range(NT):
                nc.tensor.matmul(ot[:],
                                 r(vaug[:, jt, hl * VD:(hl + 1) * VD]),
                                 r(ats[jt // 2][:, jt % 2, :]),
                                 start=(jt == 0), stop=(jt == NT - 1))
            nc.vector.tensor_copy(oT_all[:, hl, :], ot[:])

        # ---- back to token-major; extract denominators ----
        opts = persist.tile([128, NT, NH * 24], F32, tag="opts")
        ovch = persist.tile([128, NT, NH * 16], F32, tag="ovch")
        for it in range(NT):
            prw = psS.tile([128, NH, VD], F32, tag="s", name=f"prw{it}")
            for hl in range(NH):
                nc.tensor.transpose(prw[:, hl, :],
                                    oT_all[:, hl, ts(it, 128)],
                                    ident[0:VD, 0:VD])
            nc.vector.tensor_copy(den[:, it, :], prw[:, :, 40])
            # normalize all 40 head-channels by 1/denom (broadcast over ch)
            nc.vector.reciprocal(rden[:, it, :], den[:, it, :])
            nc.vector.tensor_mul(
                opts[:, it].rearrange("p (h c) -> p h c", h=NH),
                prw[:, :, 0:24],
                rden[:, it, :].to_broadcast((128, NH, 24)))
            nc.vector.tensor_mul(
                ovch[:, it].rearrange("p (h c) -> p h c", h=NH),
                prw[:, :, 24:40],
                rden[:, it, :].to_broadcast((128, NH, 16)))

        # ---- pass B: a = exp(S + ln(1/d)) row-major, DMA out ----
        nlnd = persist.tile([128, NT, NH], F32, tag="nlnd")
        nc.scalar.activation(nlnd[:], rden[:], AF.Ln)
        for it in range(NT):
            ew = epool.tile([128, NH, 512], F32, tag="e")
            for pr in range(NH // 2):
                S = psS.tile([128, 2, 512], F32, tag="s")
                for k in range(2):
                    hl = 2 * pr + k
                    half, hh = hl // 3, (hl % 3) * FD
                    nc.tensor.matmul(S[:, k, :],
                                     r(qfT[half][hh:hh + FD, ts(it, 128)]),
                                     r(kfT[half][hh:hh + FD, :]),
                                     start=True, stop=True)
                for k in range(2):
                    hl = 2 * pr + k
                    nc.scalar.activation(ew[:, hl, :], S[:, k, :], AF.Exp,
                                         bias=nlnd[:, it, hl:hl + 1])
            nc.sync.dma_start(
                a_out[0:3, it * 128:(it + 1) * 128, :]
                .rearrange("h i j -> i h j"), ew[:, 0:3, :])
            nc.gpsimd.dma_start(
                a_out[3:6, it * 128:(it + 1) * 128, :]
                .rearrange("h i j -> i h j"), ew[:, 3:6, :])

        # ---- inverse rigid + norms on normalized points (SBUF source) ----
        orv_all = opts.rearrange("p t (h e q) -> p t h e q",
                                 h=NH, e=3, q=PV)
        for it in range(NT):
            for hf in range(2):
                orv = orv_all[:, it, 3 * hf:3 * hf + 3]
                base = hf * 96
                for d in range(3):
                    o = optfin[:, it, base + d * 24:base + (d + 1) * 24] \
                        .rearrange("p (h q) -> p h q", h=3)
                    nc.vector.tensor_scalar(
                        o, orv[:, :, 0, :],
                        raugd[:, it, d:d + 1],
                        raugd[:, it, 12 + d:13 + d], OP.mult, OP.subtract)
                    for e in (1, 2):
                        nc.vector.scalar_tensor_tensor(
                            o, orv[:, :, e, :],
                            raugd[:, it, 3 * e + d:3 * e + d + 1], o,
                            OP.mult, OP.add)
                n1 = work.tile([128, 24], F32, tag="n1")
                n2 = work.tile([128, 24], F32, tag="n2")
                nc.vector.tensor_mul(n1[:], optfin[:, it, base:base + 24],
                                     optfin[:, it, base:base + 24])
                nc.vector.tensor_mul(n2[:],
                                     optfin[:, it, base + 24:base + 48],
                                     optfin[:, it, base + 24:base + 48])
                nc.vector.tensor_add(n1[:], n1[:], n2[:])
                nc.vector.tensor_mul(n2[:],
                                     optfin[:, it, base + 48:base + 72],
                                     optfin[:, it, base + 48:base + 72])
                nc.vector.tensor_add(n1[:], n1[:], n2[:])
                nc.scalar.activation(optfin[:, it, base + 72:base + 96],
                                     n1[:], AF.Sqrt, bias=eps_t[:])
                p = ptr.tile([128, 128], F32, tag="pm")
                nc.tensor.transpose(
                    p[0:96, :], optfin[:, it, base:base + 96], ident[:])
                dst = optT_a if hf == 0 else optT_b
                nc.vector.tensor_copy(dst[:, ts(it, 128)], p[0:96, :])
            # normalized o channels back to feature-major for the final mm
            p = ptr.tile([128, 128], F32, tag="pm")
            nc.tensor.transpose(
                p[0:96, :],
                ovch[:, it],
                ident[:])
            nc.vector.tensor_copy(oGrp[:, ts(it, 128)], p[0:96, :])

        # ---- final projection: s_upd partial = cat^T chunks @ Wout rows ----
        for it in range(NT):
            sp = pm.tile([128, 512], F32, tag="pm")
            nc.tensor.matmul(sp[:, 0:CS], r(oGrp[:, ts(it, 128)]), r(Wo[:]),
                             start=True, stop=False)
            nc.tensor.matmul(sp[:, 0:CS], r(optT_a[:, ts(it, 128)]),
                             r(Wx[:]), start=False, stop=False)
            nc.tensor.matmul(sp[:, 0:CS], r(optT_b[:, ts(it, 128)]),
                             r(Wn[:]), start=False, stop=True)
            sb = work.tile([128, CS], F32, tag="sb")
            nc.scalar.copy(sb[:], sp[:, 0:CS])
            nc.sync.dma_start(sup[it * 128:(it + 1) * 128, :], sb[:])


def _build():
    nc = bacc.Bacc("TRN2", target_bir_lowering=False, debug=False,
                   num_devices=N_CORES)
    din = {}
    for name, shape in [
        ("raugd", [LD, 16]), ("raugs", [LS, 16]),
        ("btile", [1, 72]), ("gamrow", [1, NH]),
    ]:
        din[name] = nc.dram_tensor(name, shape, F32, kind="ExternalInput").ap()
    for name, shape in [
        ("sdT", [CS, LD]), ("ssT", [CS, LS]),
        ("Wd", [CS, 256]), ("Ws", [CS, 408]),
        ("Wo", [96, CS]), ("Wx", [96, CS]), ("Wn", [96, CS]),
    ]:
        din[name] = nc.dram_tensor(name, shape, F32R,
                                   kind="ExternalInput").ap()
    a_out = nc.dram_tensor("a_out", [NH, LD, LS], F32,
                           kind="ExternalOutput").ap()
    sup = nc.dram_tensor("sup", [LD, CS], F32, kind="ExternalOutput").ap()

    with tile.TileContext(nc) as tc:
        _emit(tc, din, a_out, sup)
    nc.compile()
    return nc


_NC = None


def _get_nc():
    global _NC
    if _NC is None:
        _NC = _build()
    return _NC


def _prep_core(inputs, b, g):
    hs = slice(g * NH, (g + 1) * NH)
    hw = np.log1p(np.exp(inputs["head_weights"].astype(np.float64)))[hs]
    gam = (0.5 * W_L * W_C * hw).astype(np.float32)           # [NH]

    Wq = inputs["Wq"][:, g * NH * CH:(g + 1) * NH * CH] * ALPHA
    Wkv = inputs["Wkv"]
    idx_k = np.array([h * 2 * CH + c
                      for h in range(g * NH, (g + 1) * NH)
                      for c in range(CH)])
    Wk = Wkv[:, idx_k]
    Wv = Wkv[:, idx_k + CH]

    idx_qp = np.array([d * H * PQ + h * PQ + p
                       for d in range(3) for h in range(g * NH, (g + 1) * NH)
                       for p in range(PQ)])
    Wqp = inputs["Wqp"][:, idx_qp]                            # [CS, 72]
    npts = PQ + PV
    idx_kvp = np.array([d * H * npts + h * npts + p
                        for d in range(3) for h in range(g * NH, (g + 1) * NH)
                        for p in range(npts)])
    Wkvp = inputs["Wkvp"][:, idx_kvp]                         # [CS, 216]

    R_d, t_d = inputs["R_dst"][b], inputs["t_dst"][b]
    R_s, t_s = inputs["R_src"][b], inputs["t_src"][b]
    tinv = np.einsum("led,le->ld", R_d, t_d)                  # R^T t
    raugd = np.concatenate([R_d.reshape(LD, 9), t_d, tinv,
                            np.zeros((LD, 1), np.float32)], axis=1)
    raugs = np.concatenate([R_s.reshape(LS, 9), t_s,
                            np.zeros((LS, 4), np.float32)], axis=1)

    beta = 2.0 * gam
    btile = np.tile(beta[None, :, None], (3, 1, PQ)).reshape(1, 72)
    gamrow = (-gam)[None, :]

    Wout = inputs["Wout"]
    rows_o = np.arange(g * NH * CH, (g + 1) * NH * CH)
    blk = H * PV
    rows_A = np.concatenate([H * CH + k * blk + g * NH * PV
                             + np.arange(24) for k in range(4)])
    rows_B = rows_A + 24

    f32 = np.float32
    return {
        "sdT": np.ascontiguousarray(inputs["s_dst"][b].T, dtype=f32),
        "ssT": np.ascontiguousarray(inputs["s_src"][b].T, dtype=f32),
        "Wd": np.ascontiguousarray(np.concatenate(
            [Wq, Wqp, np.zeros((CS, 88), np.float32)], 1), dtype=f32),
        "Ws": np.ascontiguousarray(np.concatenate([Wk, Wv, Wkvp], 1),
                                   dtype=f32),
        "raugd": np.ascontiguousarray(raugd, dtype=f32),
        "raugs": np.ascontiguousarray(raugs, dtype=f32),
        "btile": np.ascontiguousarray(btile, dtype=f32),
        "gamrow": np.ascontiguousarray(gamrow, dtype=f32),
        "Wo": np.ascontiguousarray(Wout[rows_o], dtype=f32),
        "Wx": np.ascontiguousarray(Wout[rows_A], dtype=f32),
        "Wn": np.ascontiguousarray(Wout[rows_B], dtype=f32),
    }


def kernel(**inputs):
    nc = _get_nc()
    in_maps = [_prep_core(inputs, c // 2, c % 2) for c in range(N_CORES)]
    res = run_bass_kernel_spmd(nc, in_maps, list(range(N_CORES)))

    s_upd = np.zeros((B, LD, CS), np.float32)
    a = np.zeros((B, H, LD, LS), np.float32)
    for c in range(N_CORES):
        b, g = c // 2, c % 2
        s_upd[b] += res.results[c]["sup"]
        a[b, g * NH:(g + 1) * NH] = res.results[c]["a_out"]
    s_upd += np.asarray(inputs["b_out"], np.float32)[None, None, :]
    return s_upd, a


# revision 46
# speedup vs baseline: 1.8999x; 1.1728x over previous
"""Cross-Invariant-Point-Attention TRN2 kernel.

Sharding: 8 cores = 4 batches x 2 head-halves (6 heads each). Each core
computes its (batch, head-half) attention fully on-chip; the host sums the
two s_upd partials per batch (Wout is row-split by head) and concatenates
the per-half attention outputs. No device collectives.

Per-head feature vectors are augmented to 30 dims so every softmax bias
term rides inside the QK matmul:
  Qf = [alpha*q ; 2*gam_h*q_pts ; -gam_h ; -gam_h*|q_pts|^2 (- ln denom)]
  Kf = [k       ; k_pts         ; |k_pts|^2 ; 1]
Pass 1 computes S = Qf.Kf row-major, exp with accum_out -> denominators
(softmax needs no max-subtraction: logits are bounded), and writes the
normalized attention output. Pass 2 folds -ln(denom) into Qf and computes
exp(S^T) = normalized a^T directly, which feeds o = a @ v with v as the
stationary operand - the [512,512] attention matrix is never transposed.

Self-contained: hardcoded shapes, no sibling imports.
"""

import math

import numpy as np

import concourse.bass as bass
import concourse.tile as tile
from concourse.tile import add_dep_helper
from concourse import bacc, mybir
from concourse.bass_utils import run_bass_kernel_spmd
from concourse.masks import make_identity

F32 = mybir.dt.float32
F32R = mybir.dt.float32r
AF = mybir.ActivationFunctionType
OP = mybir.AluOpType
AX = mybir.AxisListType

B, LD, LS, CS, CH, H, PQ, PV = 4, 512, 512, 384, 16, 12, 4, 8
NH = H // 2              # local heads per core
EPS = 1e-8
W_C = math.sqrt(2.0 / (9.0 * PQ))
W_L = math.sqrt(0.5)
ALPHA = W_L / math.sqrt(CH)
N_CORES = 8
NT = LD // 128           # 4 token tiles
KT = CS // 128           # 3 contraction tiles
FD = 32                  # feature dims per head (16+12+2 bias, 2 zero pad)
VD = 42                  # per head: 24 v_pts + 16 v + ones row + zero pad


def _emit(tc, din, a_out, sup):
    nc = tc.nc
    from contextlib import ExitStack
    ctx = ExitStack()
    with ctx:
        persist = ctx.enter_context(tc.tile_pool(name="persist", bufs=1))
        work = ctx.enter_context(tc.tile_pool(name="work", bufs=4))
        epool = ctx.enter_context(tc.tile_pool(name="epool", bufs=3))
        atpool = ctx.enter_context(tc.tile_pool(name="atpool", bufs=8))
        psS = ctx.enter_context(tc.tile_pool(name="psS", bufs=2, space="PSUM"))
        pm = ctx.enter_context(tc.tile_pool(name="pm", bufs=4, space="PSUM"))
        ptr = pm
        pot = pm

        ts = bass.ts
        FR = mybir.dt.float32r

        def r(ap):
            return ap.bitcast(FR)

        # ---- constants / inputs to SBUF (DMAs spread across engine queues,
        # src-side first: the k feature path is the critical path) ----
        ident = persist.tile([128, 128], F32, tag="ident")
        make_identity(nc, ident)
        eps_t = persist.tile([128, 1], F32, tag="eps_t")
        nc.gpsimd.memset(eps_t[:], EPS)
        ones_t = persist.tile([128, NH], F32, tag="ones_t")
        nc.gpsimd.memset(ones_t[:], 1.0)

        def load(eng, name, shape, pat, dt=F32, **kw):
            t = persist.tile(shape, dt, tag=name)
            eng.dma_start(t[:], din[name].rearrange(pat, **kw))
            return t

        def load_bcast(name, cols):
            t = persist.tile([128, cols], F32, tag=name)
            src = din[name]
            nc.gpsimd.dma_start(
                t[:], bass.AP(tensor=src.tensor, offset=src.offset,
                              ap=[[0, 128]] + list(src.ap[1:])))
            return t

        Ws = load(nc.gpsimd, "Ws", [128, KT, 408], "(t p) n -> p t n", dt=F32R, p=128)
        Wd = load(nc.gpsimd, "Wd", [128, KT, 256], "(t p) n -> p t n", dt=F32R, p=128)
        ssT = persist.tile([128, KT, LS], F32R, tag="ssT")
        sdT = persist.tile([128, KT, LD], F32R, tag="sdT")
        for it in range(NT):
            nc.sync.dma_start(
                ssT[:, :, ts(it, 128)],
                din["ssT"].rearrange("(t p) n -> p t n", p=128)
                [:, :, ts(it, 128)])
        for it in range(NT):
            nc.scalar.dma_start(
                sdT[:, :, ts(it, 128)],
                din["sdT"].rearrange("(t p) n -> p t n", p=128)
                [:, :, ts(it, 128)])
        raugs = load(nc.gpsimd, "raugs", [128, NT, 16], "(t p) c -> p t c",
                     p=128)
        raugd = load(nc.gpsimd, "raugd", [128, NT, 16], "(t p) c -> p t c",
                     p=128)
        btile = load_bcast("btile", 72)
        gamrow = load_bcast("gamrow", NH)     # holds -gam_h

        Wo = persist.tile([96, CS], F32R, tag="Wo")
        nc.sync.dma_start(Wo[:], din["Wo"][:])
        Wx = persist.tile([96, CS], F32R, tag="Wx")
        nc.sync.dma_start(Wx[:], din["Wx"][:])
        Wn = persist.tile([96, CS], F32R, tag="Wn")
        nc.sync.dma_start(Wn[:], din["Wn"][:])

        # ---- persistent intermediates ----
        dstfeat = persist.tile([128, NT, 168], F32, tag="dstfeat")
        srcfeat = persist.tile([128, NT, 408], F32, tag="srcfeat")
        qp = persist.tile([128, NT, 72], F32, tag="qp")
        kvp = persist.tile([128, NT, 216], F32, tag="kvp")
        q2r = persist.tile([128, NT, NH], F32, tag="q2r")
        q2g = persist.tile([128, NT, NH], F32, tag="q2g")
        k2r = persist.tile([128, NT, NH], F32, tag="k2r")
        den = persist.tile([128, NT, NH], F32, tag="den")
        rden = persist.tile([128, NT, NH], F32, tag="rden")
        qfeat = persist.tile([128, NT, NH * FD], F32, tag="qfeat")
        kfeat = persist.tile([128, NT, NH * FD], F32, tag="kfeat")
        vaug = persist.tile([128, NT, NH * VD], F32R, tag="vaug")
        sqq = persist.tile([128, NT, 72], F32, tag="sqq")
        ksq = persist.tile([128, NT, 72], F32, tag="ksq")
        qfT = persist.tile([96, 2, LD], F32R, tag="qfT")
        kfT = persist.tile([96, 2, LS], F32R, tag="kfT")
        oT_all = persist.tile([VD, NH, LD], F32, tag="oT_all")
        oGrp = persist.tile([96, LD], F32R, tag="oGrp")
        optfin = persist.tile([128, NT, 192], F32, tag="optfin")
        optT_a = persist.tile([96, LD], F32R, tag="optT_a")
        optT_b = persist.tile([96, LD], F32R, tag="optT_b")

        qf = qfeat.rearrange("p t (h f) -> p t h f", h=NH)
        kf = kfeat.rearrange("p t (h f) -> p t h f", h=NH)
        va = vaug.rearrange("p t (h f) -> p t h f", h=NH)

        def transpose_feats(src_t, dsts, its=range(NT)):
            for it in its:
                for half in range(2):
                    p = ptr.tile([128, 128], F32, tag="pm")
                    nc.tensor.transpose(
                        p[0:96, :], src_t[:, it, half * 96:half * 96 + 96],
                        ident[:])
                    nc.vector.tensor_copy(dsts[half][:, ts(it, 128)],
                                          p[0:96, :])

        # ================= SRC (k/v) side first =================
        for it in range(NT):
            ps = pm.tile([128, 512], F32, tag="pm")
            for kt in range(KT):
                nc.tensor.matmul(ps[:, 0:408], r(ssT[:, kt, ts(it, 128)]),
                                 r(Ws[:, kt, :]), start=(kt == 0),
                                 stop=(kt == KT - 1))
            nc.scalar.copy(srcfeat[:, it, :], ps[:, 0:408])
            # rigid apply on gpsimd (overlaps q-side DVE work later)
            for d in range(3):
                o = kvp[:, it, d * 72:(d + 1) * 72]
                nc.vector.tensor_scalar(
                    o, srcfeat[:, it, 192:264],
                    raugs[:, it, 3 * d:3 * d + 1],
                    raugs[:, it, 9 + d:10 + d], OP.mult, OP.add)
                for e in (1, 2):
                    nc.vector.scalar_tensor_tensor(
                        o, srcfeat[:, it, 192 + e * 72:264 + e * 72],
                        raugs[:, it, 3 * d + e:3 * d + e + 1], o,
                        OP.mult, OP.add)
            kv = kvp[:, it, :].rearrange("p (d h q) -> p h d q",
                                         d=3, h=NH, q=PQ + PV)
            nc.vector.tensor_mul(
                ksq[:, it, :].rearrange("p (h d q) -> p h d q",
                                        h=NH, d=3, q=PQ),
                kv[:, :, :, 0:PQ], kv[:, :, :, 0:PQ])
            nc.vector.tensor_reduce(
                k2r[:, it, :],
                ksq[:, it, :].rearrange("p (h dq) -> p h dq", h=NH),
                axis=AX.X, op=OP.add)
            nc.vector.tensor_copy(
                kf[:, it, :, 0:16],
                srcfeat[:, it, 0:96].rearrange("p (h c) -> p h c", h=NH))
            nc.vector.tensor_copy(
                kf[:, it, :, 16:28].rearrange("p h (d q) -> p h d q",
                                              d=3, q=PQ),
                kv[:, :, :, 0:PQ])
            nc.vector.tensor_copy(kf[:, it, :, 28], k2r[:, it, :])
            nc.gpsimd.memset(kf[:, it, :, 29], 1.0)
            nc.gpsimd.memset(kf[:, it, :, 30:32], 0.0)
            nc.scalar.copy(
                va[:, it, :, 24:40],
                srcfeat[:, it, 96:192].rearrange("p (h c) -> p h c", h=NH))
            nc.vector.tensor_copy(
                va[:, it, :, 0:24].rearrange("p h (d q) -> p h d q",
                                             d=3, q=PV),
                kv[:, :, :, PQ:PQ + PV])
            nc.vector.tensor_copy(va[:, it, :, 40], ones_t[:])
            nc.vector.tensor_scalar(va[:, it, :, 41], ones_t[:], 0.0, None,
                                    OP.mult)
            transpose_feats(kfeat, kfT, its=[it])

        # ================= DST (q) side =================
        bt = btile.rearrange("p (d h q) -> p h d q", d=3, h=NH, q=PQ)
        for it in range(NT):
            pd = pm.tile([128, 512], F32, tag="pm")
            for kt in range(KT):
                nc.tensor.matmul(pd[:, 0:256], r(sdT[:, kt, ts(it, 128)]),
                                 r(Wd[:, kt, :]), start=(kt == 0),
                                 stop=(kt == KT - 1))
            nc.scalar.copy(dstfeat[:, it, :], pd[:, 0:168])
            for d in range(3):
                o = qp[:, it, d * 24:(d + 1) * 24]
                nc.vector.tensor_scalar(
                    o, dstfeat[:, it, 96:120],
                    raugd[:, it, 3 * d:3 * d + 1],
                    raugd[:, it, 9 + d:10 + d], OP.mult, OP.add)
                for e in (1, 2):
                    nc.vector.scalar_tensor_tensor(
                        o, dstfeat[:, it, 96 + e * 24:120 + e * 24],
                        raugd[:, it, 3 * d + e:3 * d + e + 1], o,
                        OP.mult, OP.add)
            nc.vector.tensor_mul(sqq[:, it, :], qp[:, it, :], qp[:, it, :])
            nc.vector.tensor_reduce(
                q2r[:, it, :],
                sqq[:, it, :].rearrange("p (d h q) -> p h d q",
                                        d=3, h=NH, q=PQ),
                axis=AX.XY, op=OP.add)
            nc.vector.tensor_copy(
                qf[:, it, :, 0:16],
                dstfeat[:, it, 0:96].rearrange("p (h c) -> p h c", h=NH))
            nc.vector.tensor_mul(
                qf[:, it, :, 16:28].rearrange("p h (d q) -> p h d q",
                                              d=3, q=PQ),
                qp[:, it, :].rearrange("p (d h q) -> p h d q",
                                       d=3, h=NH, q=PQ),
                bt)
            for hl in range(NH):
                nc.vector.tensor_scalar(
                    q2g[:, it, hl:hl + 1], q2r[:, it, hl:hl + 1],
                    gamrow[:, hl:hl + 1], None, OP.mult)
                nc.vector.tensor_scalar(
                    qf[:, it, hl, 28:29], gamrow[:, hl:hl + 1],
                    1.0, None, OP.mult)
            nc.vector.tensor_copy(qf[:, it, :, 29], q2g[:, it, :])
            nc.gpsimd.memset(qf[:, it, :, 30:32], 0.0)
            transpose_feats(qfeat, qfT, its=[it])

        # ---- pass A: unnormalized a^T = exp(S^T); o^T_u = vaug^T . a^T ----
        # vaug carries a ones-row per head, so o^T_u row 40 is the softmax
        # denominator for free.
        for hl in range(NH):
            half, hh = hl // 3, (hl % 3) * FD
            ats = []
            for jp in range(NT // 2):
                St = psS.tile([128, 2, 512], F32, tag="s")
                for k in range(2):
                    jt = 2 * jp + k
                    nc.tensor.matmul(St[:, k, :],
                                     r(kfT[hh:hh + FD, half, ts(jt, 128)]),
                                     r(qfT[hh:hh + FD, half, :]),
                                     start=True, stop=True)
                at = atpool.tile([128, 2, 512], F32R, tag="at")
                nc.scalar.activation(at[:], St[:], AF.Exp)
                ats.append(at)
            ot = pot.tile([VD, 512], F32, tag="pm")
            for jt in range(NT):
                nc.tensor.matmul(ot[:],
                                 r(vaug[:, jt, hl * VD:(hl + 1) * VD]),
                                 r(ats[jt // 2][:, jt % 2, :]),
                                 start=(jt == 0), stop=(jt == NT - 1))
            nc.vector.tensor_copy(oT_all[:, hl, :], ot[:])

        # ---- back to token-major; extract denominators ----
        opts = persist.tile([128, NT, NH * 24], F32, tag="opts")
        ovch = persist.tile([128, NT, NH * 16], F32, tag="ovch")
        for it in range(NT):
            prw = psS.tile([128, NH, VD], F32, tag="s", name=f"prw{it}")
            for hl in range(NH):
                nc.tensor.transpose(prw[:, hl, :],
                                    oT_all[:, hl, ts(it, 128)],
                                    ident[0:VD, 0:VD])
            nc.vector.tensor_copy(den[:, it, :], prw[:, :, 40])
            # normalize all 40 head-channels by 1/denom (broadcast over ch)
            nc.vector.reciprocal(rden[:, it, :], den[:, it, :])
            nc.vector.tensor_mul(
                opts[:, it].rearrange("p (h c) -> p h c", h=NH),
                prw[:, :, 0:24],
                rden[:, it, :].to_broadcast((128, NH, 24)))
            nc.vector.tensor_mul(
                ovch[:, it].rearrange("p (h c) -> p h c", h=NH),
                prw[:, :, 24:40],
                rden[:, it, :].to_broadcast((128, NH, 16)))

        # ---- pass B: a = exp(S + ln(1/d)) row-major, DMA out ----
        nlnd = persist.tile([128, NT, NH], F32, tag="nlnd")
        nc.scalar.activation(nlnd[:], rden[:], AF.Ln)
        for it in range(NT):
            ew = epool.tile([128, NH, 512], F32, tag="e")
            for pr in range(NH // 2):
                S = psS.tile([128, 2, 512], F32, tag="s")
                for k in range(2):
                    hl = 2 * pr + k
                    half, hh = hl // 3, (hl % 3) * FD
                    nc.tensor.matmul(S[:, k, :],
                                     r(qfT[hh:hh + FD, half, ts(it, 128)]),
                                     r(kfT[hh:hh + FD, half, :]),
                                     start=True, stop=True)
                for k in range(2):
                    hl = 2 * pr + k
                    last_expB = nc.scalar.activation(
                        ew[:, hl, :], S[:, k, :], AF.Exp,
                        bias=nlnd[:, it, hl:hl + 1])
            nc.sync.dma_start(
                a_out[0:3, it * 128:(it + 1) * 128, :]
                .rearrange("h i j -> i h j"), ew[:, 0:3, :])
            nc.gpsimd.dma_start(
                a_out[3:6, it * 128:(it + 1) * 128, :]
                .rearrange("h i j -> i h j"), ew[:, 3:6, :])

        # ---- inverse rigid + norms on normalized points (SBUF source) ----
        orv_all = opts.rearrange("p t (h e q) -> p t h e q",
                                 h=NH, e=3, q=PV)
        for it in range(NT):
            for hf in range(2):
                orv = orv_all[:, it, 3 * hf:3 * hf + 3]
                base = hf * 96
                for d in range(3):
                    o = optfin[:, it, base + d * 24:base + (d + 1) * 24] \
                        .rearrange("p (h q) -> p h q", h=3)
                    nc.vector.tensor_scalar(
                        o, orv[:, :, 0, :],
                        raugd[:, it, d:d + 1],
                        raugd[:, it, 12 + d:13 + d], OP.mult, OP.subtract)
                    for e in (1, 2):
                        nc.vector.scalar_tensor_tensor(
                            o, orv[:, :, e, :],
                            raugd[:, it, 3 * e + d:3 * e + d + 1], o,
                            OP.mult, OP.add)
                n1 = work.tile([128, 24], F32, tag="n1")
                n2 = work.tile([128, 24], F32, tag="n2")
                nc.vector.tensor_mul(n1[:], optfin[:, it, base:base + 24],
                                     optfin[:, it, base:base + 24])
                nc.vector.tensor_mul(n2[:],
                                     optfin[:, it, base + 24:base + 48],
                                     optfin[:, it, base + 24:base + 48])
                nc.vector.tensor_add(n1[:], n1[:], n2[:])
                nc.vector.tensor_mul(n2[:],
                                     optfin[:, it, base + 48:base + 72],
                                     optfin[:, it, base + 48:base + 72])
                nc.vector.tensor_add(n1[:], n1[:], n2[:])
                sq_i = nc.scalar.activation(
                    optfin[:, it, base + 72:base + 96],
                    n1[:], AF.Sqrt, bias=eps_t[:])
                add_dep_helper(sq_i.ins, last_expB.ins, sync=False,
                               reason="keep Sqrt after Exp block (ACT table)")
                p = ptr.tile([128, 128], F32, tag="pm")
                nc.tensor.transpose(
                    p[0:96, :], optfin[:, it, base:base + 96], ident[:])
                dst = optT_a if hf == 0 else optT_b
                nc.vector.tensor_copy(dst[:, ts(it, 128)], p[0:96, :])
            # normalized o channels back to feature-major for the final mm
            p = ptr.tile([128, 128], F32, tag="pm")
            nc.tensor.transpose(
                p[0:96, :],
                ovch[:, it],
                ident[:])
            nc.vector.tensor_copy(oGrp[:, ts(it, 128)], p[0:96, :])

        # ---- final projection: s_upd partial = cat^T chunks @ Wout rows ----
        for it in range(NT):
            sp = pm.tile([128, 512], F32, tag="pm")
            nc.tensor.matmul(sp[:, 0:CS], r(oGrp[:, ts(it, 128)]), r(Wo[:]),
                             start=True, stop=False)
            nc.tensor.matmul(sp[:, 0:CS], r(optT_a[:, ts(it, 128)]),
                             r(Wx[:]), start=False, stop=False)
            nc.tensor.matmul(sp[:, 0:CS], r(optT_b[:, ts(it, 128)]),
                             r(Wn[:]), start=False, stop=True)
            sb = work.tile([128, CS], F32, tag="sb")
            nc.scalar.copy(sb[:], sp[:, 0:CS])
            nc.sync.dma_start(sup[it * 128:(it + 1) * 128, :], sb[:])


def _build():
    nc = bacc.Bacc("TRN2", target_bir_lowering=False, debug=False,
                   num_devices=N_CORES)
    din = {}
    for name, shape in [
        ("raugd", [LD, 16]), ("raugs", [LS, 16]),
        ("btile", [1, 72]), ("gamrow", [1, NH]),
    ]:
        din[name] = nc.dram_tensor(name, shape, F32, kind="ExternalInput").ap()
    for name, shape in [
        ("sdT", [CS, LD]), ("ssT", [CS, LS]),
        ("Wd", [CS, 256]), ("Ws", [CS, 408]),
        ("Wo", [96, CS]), ("Wx", [96, CS]), ("Wn", [96, CS]),
    ]:
        din[name] = nc.dram_tensor(name, shape, F32R,
                                   kind="ExternalInput").ap()
    a_out = nc.dram_tensor("a_out", [NH, LD, LS], F32,
                           kind="ExternalOutput").ap()
    sup = nc.dram_tensor("sup", [LD, CS], F32, kind="ExternalOutput").ap()

    with tile.TileContext(nc) as tc:
        _emit(tc, din, a_out, sup)
    nc.compile()
    return nc


_NC = None


def _get_nc():
    global _NC
    if _NC is None:
        _NC = _build()
    return _NC


def _prep_core(inputs, b, g):
    hs = slice(g * NH, (g + 1) * NH)
    hw = np.log1p(np.exp(inputs["head_weights"].astype(np.float64)))[hs]
    gam = (0.5 * W_L * W_C * hw).astype(np.float32)           # [NH]

    Wq = inputs["Wq"][:, g * NH * CH:(g + 1) * NH * CH] * ALPHA
    Wkv = inputs["Wkv"]
    idx_k = np.array([h * 2 * CH + c
                      for h in range(g * NH, (g + 1) * NH)
                      for c in range(CH)])
    Wk = Wkv[:, idx_k]
    Wv = Wkv[:, idx_k + CH]

    idx_qp = np.array([d * H * PQ + h * PQ + p
                       for d in range(3) for h in range(g * NH, (g + 1) * NH)
                       for p in range(PQ)])
    Wqp = inputs["Wqp"][:, idx_qp]                            # [CS, 72]
    npts = PQ + PV
    idx_kvp = np.array([d * H * npts + h * npts + p
                        for d in range(3) for h in range(g * NH, (g + 1) * NH)
                        for p in range(npts)])
    Wkvp = inputs["Wkvp"][:, idx_kvp]                         # [CS, 216]

    R_d, t_d = inputs["R_dst"][b], inputs["t_dst"][b]
    R_s, t_s = inputs["R_src"][b], inputs["t_src"][b]
    tinv = np.einsum("led,le->ld", R_d, t_d)                  # R^T t
    raugd = np.concatenate([R_d.reshape(LD, 9), t_d, tinv,
                            np.zeros((LD, 1), np.float32)], axis=1)
    raugs = np.concatenate([R_s.reshape(LS, 9), t_s,
                            np.zeros((LS, 4), np.float32)], axis=1)

    beta = 2.0 * gam
    btile = np.tile(beta[None, :, None], (3, 1, PQ)).reshape(1, 72)
    gamrow = (-gam)[None, :]

    Wout = inputs["Wout"]
    rows_o = np.arange(g * NH * CH, (g + 1) * NH * CH)
    blk = H * PV
    rows_A = np.concatenate([H * CH + k * blk + g * NH * PV
                             + np.arange(24) for k in range(4)])
    rows_B = rows_A + 24

    f32 = np.float32
    return {
        "sdT": np.ascontiguousarray(inputs["s_dst"][b].T, dtype=f32),
        "ssT": np.ascontiguousarray(inputs["s_src"][b].T, dtype=f32),
        "Wd": np.ascontiguousarray(np.concatenate(
            [Wq, Wqp, np.zeros((CS, 88), np.float32)], 1), dtype=f32),
        "Ws": np.ascontiguousarray(np.concatenate([Wk, Wv, Wkvp], 1),
                                   dtype=f32),
        "raugd": np.ascontiguousarray(raugd, dtype=f32),
        "raugs": np.ascontiguousarray(raugs, dtype=f32),
        "btile": np.ascontiguousarray(btile, dtype=f32),
        "gamrow": np.ascontiguousarray(gamrow, dtype=f32),
        "Wo": np.ascontiguousarray(Wout[rows_o], dtype=f32),
        "Wx": np.ascontiguousarray(Wout[rows_A], dtype=f32),
        "Wn": np.ascontiguousarray(Wout[rows_B], dtype=f32),
    }


def kernel(**inputs):
    nc = _get_nc()
    in_maps = [_prep_core(inputs, c // 2, c % 2) for c in range(N_CORES)]
    res = run_bass_kernel_spmd(nc, in_maps, list(range(N_CORES)))

    s_upd = np.zeros((B, LD, CS), np.float32)
    a = np.zeros((B, H, LD, LS), np.float32)
    for c in range(N_CORES):
        b, g = c // 2, c % 2
        s_upd[b] += res.results[c]["sup"]
        a[b, g * NH:(g + 1) * NH] = res.results[c]["a_out"]
    s_upd += np.asarray(inputs["b_out"], np.float32)[None, None, :]
    return s_upd, a


# revision 47
# speedup vs baseline: 1.9329x; 1.0174x over previous
"""Cross-Invariant-Point-Attention TRN2 kernel.

Sharding: 8 cores = 4 batches x 2 head-halves (6 heads each). Each core
computes its (batch, head-half) attention fully on-chip; the host sums the
two s_upd partials per batch (Wout is row-split by head) and concatenates
the per-half attention outputs. No device collectives.

Per-head feature vectors are augmented to 30 dims so every softmax bias
term rides inside the QK matmul:
  Qf = [alpha*q ; 2*gam_h*q_pts ; -gam_h ; -gam_h*|q_pts|^2 (- ln denom)]
  Kf = [k       ; k_pts         ; |k_pts|^2 ; 1]
Pass 1 computes S = Qf.Kf row-major, exp with accum_out -> denominators
(softmax needs no max-subtraction: logits are bounded), and writes the
normalized attention output. Pass 2 folds -ln(denom) into Qf and computes
exp(S^T) = normalized a^T directly, which feeds o = a @ v with v as the
stationary operand - the [512,512] attention matrix is never transposed.

Self-contained: hardcoded shapes, no sibling imports.
"""

import math

import numpy as np

import concourse.bass as bass
import concourse.tile as tile
from concourse.tile import add_dep_helper
from concourse import bacc, mybir
from concourse.bass_utils import run_bass_kernel_spmd
from concourse.masks import make_identity

F32 = mybir.dt.float32
F32R = mybir.dt.float32r
AF = mybir.ActivationFunctionType
OP = mybir.AluOpType
AX = mybir.AxisListType

B, LD, LS, CS, CH, H, PQ, PV = 4, 512, 512, 384, 16, 12, 4, 8
NH = H // 2              # local heads per core
EPS = 1e-8
W_C = math.sqrt(2.0 / (9.0 * PQ))
W_L = math.sqrt(0.5)
ALPHA = W_L / math.sqrt(CH)
N_CORES = 8
NT = LD // 128           # 4 token tiles
KT = CS // 128           # 3 contraction tiles
FD = 32                  # feature dims per head (16+12+2 bias, 2 zero pad)
VD = 42                  # per head: 24 v_pts + 16 v + ones row + zero pad


def _emit(tc, din, a_out, sup):
    nc = tc.nc
    from contextlib import ExitStack
    ctx = ExitStack()
    with ctx:
        persist = ctx.enter_context(tc.tile_pool(name="persist", bufs=1))
        work = ctx.enter_context(tc.tile_pool(name="work", bufs=8))
        epool = ctx.enter_context(tc.tile_pool(name="epool", bufs=4))
        atpool = ctx.enter_context(tc.tile_pool(name="atpool", bufs=8))
        psS = ctx.enter_context(tc.tile_pool(name="psS", bufs=2, space="PSUM"))
        pm = ctx.enter_context(tc.tile_pool(name="pm", bufs=4, space="PSUM"))
        ptr = pm
        pot = pm

        ts = bass.ts
        FR = mybir.dt.float32r

        def r(ap):
            return ap.bitcast(FR)

        # ---- constants / inputs to SBUF (DMAs spread across engine queues,
        # src-side first: the k feature path is the critical path) ----
        ident = persist.tile([128, 128], F32, tag="ident")
        make_identity(nc, ident)
        eps_t = persist.tile([128, 1], F32, tag="eps_t")
        nc.gpsimd.memset(eps_t[:], EPS)
        ones_t = persist.tile([128, NH], F32, tag="ones_t")
        nc.gpsimd.memset(ones_t[:], 1.0)

        def load(eng, name, shape, pat, dt=F32, **kw):
            t = persist.tile(shape, dt, tag=name)
            eng.dma_start(t[:], din[name].rearrange(pat, **kw))
            return t

        def load_bcast(name, cols):
            t = persist.tile([128, cols], F32, tag=name)
            src = din[name]
            nc.gpsimd.dma_start(
                t[:], bass.AP(tensor=src.tensor, offset=src.offset,
                              ap=[[0, 128]] + list(src.ap[1:])))
            return t

        Ws = load(nc.gpsimd, "Ws", [128, KT, 408], "(t p) n -> p t n", dt=F32R, p=128)
        Wd = load(nc.gpsimd, "Wd", [128, KT, 256], "(t p) n -> p t n", dt=F32R, p=128)
        ssT = persist.tile([128, KT, LS], F32R, tag="ssT")
        sdT = persist.tile([128, KT, LD], F32R, tag="sdT")
        for it in range(NT):
            nc.sync.dma_start(
                ssT[:, :, ts(it, 128)],
                din["ssT"].rearrange("(t p) n -> p t n", p=128)
                [:, :, ts(it, 128)])
        for it in range(NT):
            nc.scalar.dma_start(
                sdT[:, :, ts(it, 128)],
                din["sdT"].rearrange("(t p) n -> p t n", p=128)
                [:, :, ts(it, 128)])
        raugs = load(nc.gpsimd, "raugs", [128, NT, 16], "(t p) c -> p t c",
                     p=128)
        raugd = load(nc.gpsimd, "raugd", [128, NT, 16], "(t p) c -> p t c",
                     p=128)
        btile = load_bcast("btile", 72)
        gamrow = load_bcast("gamrow", NH)     # holds -gam_h

        Wo = persist.tile([96, CS], F32R, tag="Wo")
        nc.sync.dma_start(Wo[:], din["Wo"][:])
        Wx = persist.tile([96, CS], F32R, tag="Wx")
        nc.sync.dma_start(Wx[:], din["Wx"][:])
        Wn = persist.tile([96, CS], F32R, tag="Wn")
        nc.sync.dma_start(Wn[:], din["Wn"][:])

        # ---- persistent intermediates ----
        dstfeat = persist.tile([128, NT, 168], F32, tag="dstfeat")
        srcfeat = persist.tile([128, NT, 408], F32, tag="srcfeat")
        qp = persist.tile([128, NT, 72], F32, tag="qp")
        kvp = persist.tile([128, NT, 216], F32, tag="kvp")
        q2r = persist.tile([128, NT, NH], F32, tag="q2r")
        q2g = persist.tile([128, NT, NH], F32, tag="q2g")
        k2r = persist.tile([128, NT, NH], F32, tag="k2r")
        den = persist.tile([128, NT, NH], F32, tag="den")
        rden = persist.tile([128, NT, NH], F32, tag="rden")
        qfeat = persist.tile([128, NT, NH * FD], F32, tag="qfeat")
        kfeat = persist.tile([128, NT, NH * FD], F32, tag="kfeat")
        vaug = persist.tile([128, NT, NH * VD], F32R, tag="vaug")
        sqq = persist.tile([128, NT, 72], F32, tag="sqq")
        ksq = persist.tile([128, NT, 72], F32, tag="ksq")
        qfT = persist.tile([96, 2, LD], F32R, tag="qfT")
        kfT = persist.tile([96, 2, LS], F32R, tag="kfT")
        oT_all = persist.tile([VD, NH, LD], F32, tag="oT_all")
        oGrp = persist.tile([96, LD], F32R, tag="oGrp")
        optfin = persist.tile([128, NT, 192], F32, tag="optfin")
        optT_a = persist.tile([96, LD], F32R, tag="optT_a")
        optT_b = persist.tile([96, LD], F32R, tag="optT_b")

        qf = qfeat.rearrange("p t (h f) -> p t h f", h=NH)
        kf = kfeat.rearrange("p t (h f) -> p t h f", h=NH)
        va = vaug.rearrange("p t (h f) -> p t h f", h=NH)

        def transpose_feats(src_t, dsts, its=range(NT)):
            for it in its:
                for half in range(2):
                    p = ptr.tile([128, 128], F32, tag="pm")
                    nc.tensor.transpose(
                        p[0:96, :], src_t[:, it, half * 96:half * 96 + 96],
                        ident[:])
                    nc.vector.tensor_copy(dsts[half][:, ts(it, 128)],
                                          p[0:96, :])

        # ================= SRC (k/v) side first =================
        for it in range(NT):
            ps = pm.tile([128, 512], F32, tag="pm")
            for kt in range(KT):
                nc.tensor.matmul(ps[:, 0:408], r(ssT[:, kt, ts(it, 128)]),
                                 r(Ws[:, kt, :]), start=(kt == 0),
                                 stop=(kt == KT - 1))
            nc.scalar.copy(srcfeat[:, it, :], ps[:, 0:408])
            # rigid apply on gpsimd (overlaps q-side DVE work later)
            for d in range(3):
                o = kvp[:, it, d * 72:(d + 1) * 72]
                nc.vector.tensor_scalar(
                    o, srcfeat[:, it, 192:264],
                    raugs[:, it, 3 * d:3 * d + 1],
                    raugs[:, it, 9 + d:10 + d], OP.mult, OP.add)
                for e in (1, 2):
                    nc.vector.scalar_tensor_tensor(
                        o, srcfeat[:, it, 192 + e * 72:264 + e * 72],
                        raugs[:, it, 3 * d + e:3 * d + e + 1], o,
                        OP.mult, OP.add)
            kv = kvp[:, it, :].rearrange("p (d h q) -> p h d q",
                                         d=3, h=NH, q=PQ + PV)
            nc.vector.tensor_mul(
                ksq[:, it, :].rearrange("p (h d q) -> p h d q",
                                        h=NH, d=3, q=PQ),
                kv[:, :, :, 0:PQ], kv[:, :, :, 0:PQ])
            nc.vector.tensor_reduce(
                k2r[:, it, :],
                ksq[:, it, :].rearrange("p (h dq) -> p h dq", h=NH),
                axis=AX.X, op=OP.add)
            nc.vector.tensor_copy(
                kf[:, it, :, 0:16],
                srcfeat[:, it, 0:96].rearrange("p (h c) -> p h c", h=NH))
            nc.vector.tensor_copy(
                kf[:, it, :, 16:28].rearrange("p h (d q) -> p h d q",
                                              d=3, q=PQ),
                kv[:, :, :, 0:PQ])
            nc.vector.tensor_copy(kf[:, it, :, 28], k2r[:, it, :])
            nc.gpsimd.memset(kf[:, it, :, 29], 1.0)
            nc.gpsimd.memset(kf[:, it, :, 30:32], 0.0)
            nc.scalar.copy(
                va[:, it, :, 24:40],
                srcfeat[:, it, 96:192].rearrange("p (h c) -> p h c", h=NH))
            nc.vector.tensor_copy(
                va[:, it, :, 0:24].rearrange("p h (d q) -> p h d q",
                                             d=3, q=PV),
                kv[:, :, :, PQ:PQ + PV])
            nc.vector.tensor_copy(va[:, it, :, 40], ones_t[:])
            nc.vector.tensor_scalar(va[:, it, :, 41], ones_t[:], 0.0, None,
                                    OP.mult)
            transpose_feats(kfeat, kfT, its=[it])

        # ================= DST (q) side =================
        bt = btile.rearrange("p (d h q) -> p h d q", d=3, h=NH, q=PQ)
        for it in range(NT):
            pd = pm.tile([128, 512], F32, tag="pm")
            for kt in range(KT):
                nc.tensor.matmul(pd[:, 0:256], r(sdT[:, kt, ts(it, 128)]),
                                 r(Wd[:, kt, :]), start=(kt == 0),
                                 stop=(kt == KT - 1))
            nc.scalar.copy(dstfeat[:, it, :], pd[:, 0:168])
            for d in range(3):
                o = qp[:, it, d * 24:(d + 1) * 24]
                nc.vector.tensor_scalar(
                    o, dstfeat[:, it, 96:120],
                    raugd[:, it, 3 * d:3 * d + 1],
                    raugd[:, it, 9 + d:10 + d], OP.mult, OP.add)
                for e in (1, 2):
                    nc.vector.scalar_tensor_tensor(
                        o, dstfeat[:, it, 96 + e * 24:120 + e * 24],
                        raugd[:, it, 3 * d + e:3 * d + e + 1], o,
                        OP.mult, OP.add)
            nc.vector.tensor_mul(sqq[:, it, :], qp[:, it, :], qp[:, it, :])
            nc.vector.tensor_reduce(
                q2r[:, it, :],
                sqq[:, it, :].rearrange("p (d h q) -> p h d q",
                                        d=3, h=NH, q=PQ),
                axis=AX.XY, op=OP.add)
            nc.vector.tensor_copy(
                qf[:, it, :, 0:16],
                dstfeat[:, it, 0:96].rearrange("p (h c) -> p h c", h=NH))
            nc.vector.tensor_mul(
                qf[:, it, :, 16:28].rearrange("p h (d q) -> p h d q",
                                              d=3, q=PQ),
                qp[:, it, :].rearrange("p (d h q) -> p h d q",
                                       d=3, h=NH, q=PQ),
                bt)
            for hl in range(NH):
                nc.vector.tensor_scalar(
                    q2g[:, it, hl:hl + 1], q2r[:, it, hl:hl + 1],
                    gamrow[:, hl:hl + 1], None, OP.mult)
                nc.vector.tensor_scalar(
                    qf[:, it, hl, 28:29], gamrow[:, hl:hl + 1],
                    1.0, None, OP.mult)
            nc.vector.tensor_copy(qf[:, it, :, 29], q2g[:, it, :])
            nc.gpsimd.memset(qf[:, it, :, 30:32], 0.0)
            transpose_feats(qfeat, qfT, its=[it])

        # ---- pass A: unnormalized a^T = exp(S^T); o^T_u = vaug^T . a^T ----
        # vaug carries a ones-row per head, so o^T_u row 40 is the softmax
        # denominator for free.
        for hl in range(NH):
            half, hh = hl // 3, (hl % 3) * FD
            ats = []
            for jp in range(NT // 2):
                St = psS.tile([128, 2, 512], F32, tag="s")
                for k in range(2):
                    jt = 2 * jp + k
                    nc.tensor.matmul(St[:, k, :],
                                     r(kfT[hh:hh + FD, half, ts(jt, 128)]),
                                     r(qfT[hh:hh + FD, half, :]),
                                     start=True, stop=True)
                at = atpool.tile([128, 2, 512], F32R, tag="at")
                nc.scalar.activation(at[:], St[:], AF.Exp)
                ats.append(at)
            ot = pot.tile([VD, 512], F32, tag="pm")
            for jt in range(NT):
                nc.tensor.matmul(ot[:],
                                 r(vaug[:, jt, hl * VD:(hl + 1) * VD]),
                                 r(ats[jt // 2][:, jt % 2, :]),
                                 start=(jt == 0), stop=(jt == NT - 1))
            nc.vector.tensor_copy(oT_all[:, hl, :], ot[:])

        # ---- back to token-major; extract denominators ----
        opts = persist.tile([128, NT, NH * 24], F32, tag="opts")
        ovch = persist.tile([128, NT, NH * 16], F32, tag="ovch")
        for it in range(NT):
            prw = psS.tile([128, NH, VD], F32, tag="s", name=f"prw{it}")
            for hl in range(NH):
                nc.tensor.transpose(prw[:, hl, :],
                                    oT_all[:, hl, ts(it, 128)],
                                    ident[0:VD, 0:VD])
            nc.vector.tensor_copy(den[:, it, :], prw[:, :, 40])
            # normalize all 40 head-channels by 1/denom (broadcast over ch)
            nc.vector.reciprocal(rden[:, it, :], den[:, it, :])
            nc.vector.tensor_mul(
                opts[:, it].rearrange("p (h c) -> p h c", h=NH),
                prw[:, :, 0:24],
                rden[:, it, :].to_broadcast((128, NH, 24)))
            nc.vector.tensor_mul(
                ovch[:, it].rearrange("p (h c) -> p h c", h=NH),
                prw[:, :, 24:40],
                rden[:, it, :].to_broadcast((128, NH, 16)))

        # ---- pass B: a = exp(S + ln(1/d)) row-major, DMA out ----
        nlnd = persist.tile([128, NT, NH], F32, tag="nlnd")
        nc.scalar.activation(nlnd[:], rden[:], AF.Ln)
        for it in range(NT):
            ew = epool.tile([128, NH, 512], F32, tag="e")
            for pr in range(NH // 2):
                S = psS.tile([128, 2, 512], F32, tag="s")
                for k in range(2):
                    hl = 2 * pr + k
                    half, hh = hl // 3, (hl % 3) * FD
                    nc.tensor.matmul(S[:, k, :],
                                     r(qfT[hh:hh + FD, half, ts(it, 128)]),
                                     r(kfT[hh:hh + FD, half, :]),
                                     start=True, stop=True)
                for k in range(2):
                    hl = 2 * pr + k
                    last_expB = nc.scalar.activation(
                        ew[:, hl, :], S[:, k, :], AF.Exp,
                        bias=nlnd[:, it, hl:hl + 1])
            nc.sync.dma_start(
                a_out[0:3, it * 128:(it + 1) * 128, :]
                .rearrange("h i j -> i h j"), ew[:, 0:3, :])
            nc.gpsimd.dma_start(
                a_out[3:6, it * 128:(it + 1) * 128, :]
                .rearrange("h i j -> i h j"), ew[:, 3:6, :])

        # ---- inverse rigid + norms on normalized points (SBUF source) ----
        orv_all = opts.rearrange("p t (h e q) -> p t h e q",
                                 h=NH, e=3, q=PV)
        for it in range(NT):
            for hf in range(2):
                orv = orv_all[:, it, 3 * hf:3 * hf + 3]
                base = hf * 96
                for d in range(3):
                    o = optfin[:, it, base + d * 24:base + (d + 1) * 24] \
                        .rearrange("p (h q) -> p h q", h=3)
                    nc.vector.tensor_scalar(
                        o, orv[:, :, 0, :],
                        raugd[:, it, d:d + 1],
                        raugd[:, it, 12 + d:13 + d], OP.mult, OP.subtract)
                    for e in (1, 2):
                        nc.vector.scalar_tensor_tensor(
                            o, orv[:, :, e, :],
                            raugd[:, it, 3 * e + d:3 * e + d + 1], o,
                            OP.mult, OP.add)
                n1 = work.tile([128, 24], F32, tag="n1")
                n2 = work.tile([128, 24], F32, tag="n2")
                nc.vector.tensor_mul(n1[:], optfin[:, it, base:base + 24],
                                     optfin[:, it, base:base + 24])
                nc.vector.tensor_mul(n2[:],
                                     optfin[:, it, base + 24:base + 48],
                                     optfin[:, it, base + 24:base + 48])
                nc.vector.tensor_add(n1[:], n1[:], n2[:])
                nc.vector.tensor_mul(n2[:],
                                     optfin[:, it, base + 48:base + 72],
                                     optfin[:, it, base + 48:base + 72])
                nc.vector.tensor_add(n1[:], n1[:], n2[:])
                sq_i = nc.scalar.activation(
                    optfin[:, it, base + 72:base + 96],
                    n1[:], AF.Sqrt, bias=eps_t[:])
                add_dep_helper(sq_i.ins, last_expB.ins, sync=False,
                               reason="keep Sqrt after Exp block (ACT table)")
                p = ptr.tile([128, 128], F32, tag="pm")
                nc.tensor.transpose(
                    p[0:96, :], optfin[:, it, base:base + 96], ident[:])
                dst = optT_a if hf == 0 else optT_b
                nc.vector.tensor_copy(dst[:, ts(it, 128)], p[0:96, :])
            # normalized o channels back to feature-major for the final mm
            p = ptr.tile([128, 128], F32, tag="pm")
            nc.tensor.transpose(
                p[0:96, :],
                ovch[:, it],
                ident[:])
            nc.vector.tensor_copy(oGrp[:, ts(it, 128)], p[0:96, :])

        # ---- final projection: s_upd partial = cat^T chunks @ Wout rows ----
        for it in range(NT):
            sp = pm.tile([128, 512], F32, tag="pm")
            nc.tensor.matmul(sp[:, 0:CS], r(oGrp[:, ts(it, 128)]), r(Wo[:]),
                             start=True, stop=False)
            nc.tensor.matmul(sp[:, 0:CS], r(optT_a[:, ts(it, 128)]),
                             r(Wx[:]), start=False, stop=False)
            nc.tensor.matmul(sp[:, 0:CS], r(optT_b[:, ts(it, 128)]),
                             r(Wn[:]), start=False, stop=True)
            sb = work.tile([128, CS], F32, tag="sb")
            nc.scalar.copy(sb[:], sp[:, 0:CS])
            nc.sync.dma_start(sup[it * 128:(it + 1) * 128, :], sb[:])


def _build():
    nc = bacc.Bacc("TRN2", target_bir_lowering=False, debug=False,
                   num_devices=N_CORES)
    din = {}
    for name, shape in [
        ("raugd", [LD, 16]), ("raugs", [LS, 16]),
        ("btile", [1, 72]), ("gamrow", [1, NH]),
    ]:
        din[name] = nc.dram_tensor(name, shape, F32, kind="ExternalInput").ap()
    for name, shape in [
        ("sdT", [CS, LD]), ("ssT", [CS, LS]),
        ("Wd", [CS, 256]), ("Ws", [CS, 408]),
        ("Wo", [96, CS]), ("Wx", [96, CS]), ("Wn", [96, CS]),
    ]:
        din[name] = nc.dram_tensor(name, shape, F32R,
                                   kind="ExternalInput").ap()
    a_out = nc.dram_tensor("a_out", [NH, LD, LS], F32,
                           kind="ExternalOutput").ap()
    sup = nc.dram_tensor("sup", [LD, CS], F32, kind="ExternalOutput").ap()

    with tile.TileContext(nc) as tc:
        _emit(tc, din, a_out, sup)
    nc.compile()
    return nc


_NC = None


def _get_nc():
    global _NC
    if _NC is None:
        _NC = _build()
    return _NC


def _prep_core(inputs, b, g):
    hs = slice(g * NH, (g + 1) * NH)
    hw = np.log1p(np.exp(inputs["head_weights"].astype(np.float64)))[hs]
    gam = (0.5 * W_L * W_C * hw).astype(np.float32)           # [NH]

    Wq = inputs["Wq"][:, g * NH * CH:(g + 1) * NH * CH] * ALPHA
    Wkv = inputs["Wkv"]
    idx_k = np.array([h * 2 * CH + c
                      for h in range(g * NH, (g + 1) * NH)
                      for c in range(CH)])
    Wk = Wkv[:, idx_k]
    Wv = Wkv[:, idx_k + CH]

    idx_qp = np.array([d * H * PQ + h * PQ + p
                       for d in range(3) for h in range(g * NH, (g + 1) * NH)
                       for p in range(PQ)])
    Wqp = inputs["Wqp"][:, idx_qp]                            # [CS, 72]
    npts = PQ + PV
    idx_kvp = np.array([d * H * npts + h * npts + p
                        for d in range(3) for h in range(g * NH, (g + 1) * NH)
                        for p in range(npts)])
    Wkvp = inputs["Wkvp"][:, idx_kvp]                         # [CS, 216]

    R_d, t_d = inputs["R_dst"][b], inputs["t_dst"][b]
    R_s, t_s = inputs["R_src"][b], inputs["t_src"][b]
    tinv = np.einsum("led,le->ld", R_d, t_d)                  # R^T t
    raugd = np.concatenate([R_d.reshape(LD, 9), t_d, tinv,
                            np.zeros((LD, 1), np.float32)], axis=1)
    raugs = np.concatenate([R_s.reshape(LS, 9), t_s,
                            np.zeros((LS, 4), np.float32)], axis=1)

    beta = 2.0 * gam
    btile = np.tile(beta[None, :, None], (3, 1, PQ)).reshape(1, 72)
    gamrow = (-gam)[None, :]

    Wout = inputs["Wout"]
    rows_o = np.arange(g * NH * CH, (g + 1) * NH * CH)
    blk = H * PV
    rows_A = np.concatenate([H * CH + k * blk + g * NH * PV
                             + np.arange(24) for k in range(4)])
    rows_B = rows_A + 24

    f32 = np.float32
    return {
        "sdT": np.ascontiguousarray(inputs["s_dst"][b].T, dtype=f32),
        "ssT": np.ascontiguousarray(inputs["s_src"][b].T, dtype=f32),
        "Wd": np.ascontiguousarray(np.concatenate(
            [Wq, Wqp, np.zeros((CS, 88), np.float32)], 1), dtype=f32),
        "Ws": np.ascontiguousarray(np.concatenate([Wk, Wv, Wkvp], 1),
                                   dtype=f32),
        "raugd": np.ascontiguousarray(raugd, dtype=f32),
        "raugs": np.ascontiguousarray(raugs, dtype=f32),
        "btile": np.ascontiguousarray(btile, dtype=f32),
        "gamrow": np.ascontiguousarray(gamrow, dtype=f32),
        "Wo": np.ascontiguousarray(Wout[rows_o], dtype=f32),
        "Wx": np.ascontiguousarray(Wout[rows_A], dtype=f32),
        "Wn": np.ascontiguousarray(Wout[rows_B], dtype=f32),
    }


def kernel(**inputs):
    nc = _get_nc()
    in_maps = [_prep_core(inputs, c // 2, c % 2) for c in range(N_CORES)]
    res = run_bass_kernel_spmd(nc, in_maps, list(range(N_CORES)))

    s_upd = np.zeros((B, LD, CS), np.float32)
    a = np.zeros((B, H, LD, LS), np.float32)
    for c in range(N_CORES):
        b, g = c // 2, c % 2
        s_upd[b] += res.results[c]["sup"]
        a[b, g * NH:(g + 1) * NH] = res.results[c]["a_out"]
    s_upd += np.asarray(inputs["b_out"], np.float32)[None, None, :]
    return s_upd, a
